# revision 60
# baseline (speedup 1.0000x reference)
"""Causal multi-head attention block (qkv -> attention -> proj) on 8 TRN2 cores.

Problem: x[2,2048,1024], w_qkv[3072,1024], b_qkv[3072], w_proj[1024,1024],
b_proj[1024]; H=16 heads, D=64; softmax scale 1/sqrt(1024).

Sharding: core = (batch b, head-group hg); 2 batches x 4 groups of 4 heads.
Each core computes qkv for its 4 heads, causal attention, and a partial
projection (its heads' columns of w_proj); host sums the 4 partials per batch
and adds the folded bias (w_proj @ b_v + b_proj).

The kernel is ONE software-pipelined instruction stream of attention
half-slabs (one 128-row s-tile x 512 t-cols x 2 heads): QK -> exp -> PV.
The score PSUM tile [128,1024] is double-buffered, so QK(g+1) issues right
after QK(g)'s scores are consumed-by-exp begins -- the PE never waits on
the scalar engine's exp. qkv/proj matmuls of other t-chunks fill the
remaining tensor slack (deadline-rate B queue for qkv, backfill C queue for
proj) so the PE never idles into a low p-state.

Layouts: everything the PE contracts over is partition-major. q,k GEMMs run
in fp8e4m3 DoubleRow (2 k-tiles per pass, weights pre-scaled 32x, exp scale
adjusted); v is produced directly in [t, dims] (x tiles stationary) so no PE
transposes are needed; the PV stationary v-tile is [v_h (64) | ones (64)] so
PV yields the softmax denominator on partitions 64..127 for free. The two
heads' QK matmuls use PE quadrants (0,0)/(64,0), which the array overlaps
almost completely. Causality: above-diagonal s-tiles are skipped; in
diagonal half-slabs QK, exp and PV all cover only the valid column suffix
[c0:512] (PSUM accumulate over partial column ranges; stop-flag bookkeeping
relaxed via skip_group_check), and the [128,128] boundary triangles are
masked on the vector engine. Inputs arrive host-packed in SBUF layout and
are sliced across the three DMA rings by need-time, with bulk tails issued
after attention starts.

Attention-path and projection tensors are fp16 (PSUM accumulation fp32);
the per-core partial y is returned fp16 and summed on the host in fp32.
"""

import numpy as np
from contextlib import ExitStack

import concourse.bass as bass
import concourse.bacc as bacc
import concourse.tile as tile
import concourse.mybir as mybir
from concourse.bass_utils import run_bass_kernel_spmd

B, T, C, H = 2, 2048, 1024, 16
D = C // H                  # 64, head dim
HPC = 4                     # heads per core
N_CORES = 8
NT = T // 128               # 16 t-tiles / s-tiles of 128
NCT = C // 128              # 8 contraction tiles over C
TCH = T // 512              # 4 t-chunks of 512
SCALE = 1.0 / np.sqrt(np.float32(C))   # 1/32

F32 = mybir.dt.float32
F16 = mybir.dt.float16
F8 = mybir.dt.float8e4
EXP = mybir.ActivationFunctionType.Exp
DR = mybir.MatmulPerfMode.DoubleRow

VW = 2 * D                  # 128: per-head block in v_sb = [v_h (64) | ones (64)]
MM_NS = 230.0               # planning est: one N=512 fp16 matmul slot
W8 = 32.0                   # fp8 pre-scale on w_q/w_k (and b_q/b_k)

_CACHE = {}


def _build():
    """Build + compile the SPMD program (identical on all 8 cores)."""
    nc = bacc.Bacc("TRN2", target_bir_lowering=False, debug=False)

    # all inputs arrive pre-packed in SBUF layout ([128, cols]) so each is
    # one or two contiguous DMAs; x is (t-chunk, c-tile, u) chunk-major
    xp = nc.dram_tensor("xp", [128, TCH * NCT * 512], F16, kind="ExternalInput")
    xp8 = nc.dram_tensor("xp8", [128, TCH * NCT * 512], F8, kind="ExternalInput")
    wqk8 = nc.dram_tensor("wqk8", [128, NCT * 512], F8, kind="ExternalInput")  # 32*(wq|wk)
    wvp = nc.dram_tensor("wvp", [128, NCT * 256], F16, kind="ExternalInput")
    wpp = nc.dram_tensor("wpp", [128, 2 * C], F16, kind="ExternalInput")
    bqkv = nc.dram_tensor("bqkv", [128, 4], F32, kind="ExternalInput")    # 32*(bq|bk)
    mask = nc.dram_tensor("mask", [128, 128], F16, kind="ExternalInput")  # tri block
    y = nc.dram_tensor("y", [T, C], F16, kind="ExternalOutput")

    with tile.TileContext(nc) as tc, ExitStack() as ctx:
        sb = ctx.enter_context(tc.tile_pool(name="persist", bufs=1))

        # ---- persistent SBUF tensors ----
        # inputs are split into separate tiles (per chunk-half / m-tile):
        # the framework tracks hazards per tile, so fine tiles = precise
        # deps (a consumer never waits for an unrelated later DMA slice)
        w8t = [sb.tile([128, 2048], F8, tag=f"wqk8{h}", name=f"wqk8{h}")
               for h in range(2)]
        wva = sb.tile([128, 1024], F16, tag="wva")     # c-tiles 0-3 [m 256]
        wvb = sb.tile([128, 1024], F16, tag="wvb")     # c-tiles 4-7
        wp_sb = sb.tile([128, 2 * C], F16, tag="wp")               # [ci-tile][co 1024]
        bqkv_sb = sb.tile([128, 4], F32, tag="bqkv")
        mask_sb = sb.tile([128, 128], F16, tag="mask")
        xt = [[sb.tile([128, 2048], F16, tag=f"x{w}{h}", name=f"x{w}{h}")
               for h in range(2)]
              for w in range(TCH)]                     # (t-tile pair, c, u128)
        x8t = [[sb.tile([128, 2048], F8, tag=f"x8{w}{h}", name=f"x8{w}{h}")
                for h in range(2)]
               for w in range(TCH)]                    # (c-tile, u512) halves
        qkt = [[sb.tile([128, 512], F16, tag=f"qk{mt}{w}", name=f"qk{mt}{w}")
                for w in range(TCH)]
               for mt in range(4)]                     # q^T|k^T, 32x scale
        v_sb = sb.tile([128, NT * HPC * VW], F16, tag="v")  # [s-tile][h][v|ones]
        on_sb = sb.tile([128, 2 * T], F16, tag="onorm")     # O_norm^T [ci-tile][t]

        with tc.tile_pool(name="psg", bufs=2, space="PSUM") as psg, \
             tc.tile_pool(name="psacc", bufs=1, space="PSUM") as psacc, \
             tc.tile_pool(name="gem", bufs=2, space="PSUM") as gem, \
             tc.tile_pool(name="att", bufs=6) as att, \
             tc.tile_pool(name="rlp", bufs=3) as rlp, \
             tc.tile_pool(name="yst", bufs=8) as yst:

            # ---- input DMAs, strictly need-ordered. Prologue: the critical
            # 1MB (wqk8 + x8 chunk0) is interleaved c-tile-major across all
            # three rings so the first qk GEMM starts as early as possible;
            # then wv + x chunk0 arrive t-tile-major at the rate PV consumes
            # v tiles. Chunks 1-3 are emitted in stages keyed to phase
            # progress (on the idle gpsimd/sync rings only, so mid-kernel
            # triggers never stall the scalar exp stream). ----
            CH0 = NCT * 512           # cols per t-chunk in packed x
            # triggers cost ~640ns EACH on the issuing engine, so the
            # prologue uses few, large slices, spread so the first qk
            # GEMM's inputs (w8t[0], x8t[0][0], bqkv) land first
            nc.sync.dma_start(x8t[0][0][:], xp8.ap()[:, 0:2048])
            nc.scalar.dma_start(w8t[0][:], wqk8.ap()[:, 0:2048])
            nc.gpsimd.dma_start(bqkv_sb[:], bqkv.ap())
            nc.scalar.dma_start(w8t[1][:], wqk8.ap()[:, 2048:4096])
            nc.gpsimd.dma_start(x8t[0][1][:], xp8.ap()[:, 2048:4096])
            nc.sync.dma_start(xt[0][0][:], xp.ap()[:, 0:2048])
            nc.scalar.dma_start(wva[:], wvp.ap()[:, 0:1024])
            nc.gpsimd.dma_start(wvb[:], wvp.ap()[:, 1024:2048])
            nc.sync.dma_start(x8t[1][0][:], xp8.ap()[:, CH0:CH0 + 2048])
            nc.scalar.dma_start(mask_sb[:], mask.ap())   # first diag exp
            # chunk 1 rides the prologue too, ring-balanced by need time
            # (per-queue service is FIFO, so it never overtakes the
            # critical slices above)
            nc.sync.dma_start(xt[0][1][:], xp.ap()[:, 2048:4096])
            nc.sync.dma_start(x8t[1][1][:], xp8.ap()[:, CH0 + 2048:CH0 + 4096])
            nc.gpsimd.dma_start(xt[1][0][:], xp.ap()[:, CH0:CH0 + 2048])
            nc.gpsimd.dma_start(xt[1][1][:], xp.ap()[:, CH0 + 2048:CH0 + 4096])
            # ones columns of v_sb (softmax denominator trick), cols 64..127
            # per head: memset, chunk-0 s-tiles first
            vdst = v_sb[:].rearrange("p (s h e) -> p s h e", s=NT, h=HPC)[:, :, :, D:VW]
            nc.gpsimd.memset(vdst[:, 0:4], 1.0)
            nc.gpsimd.memset(vdst[:, 4:8], 1.0)

            def emit_stage(w):
                """Ship chunk w's x8 + x fp16 (+ stragglers) on idle rings."""
                for h in range(2):
                    a = w * CH0 + h * 2048
                    nc.gpsimd.dma_start(x8t[w][h][:], xp8.ap()[:, a:a + 2048])
                    nc.sync.dma_start(xt[w][h][:], xp.ap()[:, a:a + 2048])
                if w == 2:
                    nc.gpsimd.memset(vdst[:, 8:NT], 1.0)
                    nc.gpsimd.dma_start(wp_sb[:], wpp.ap())

            x8v = [[x8t[w][h][:].rearrange("p (c u) -> p c u", c=4)
                    for h in range(2)] for w in range(TCH)]
            xv = [[xt[w][h][:].rearrange("p (i c u) -> p i c u", i=2, c=NCT)
                   for h in range(2)] for w in range(TCH)]
            w8v = [w8t[h][:].rearrange("p (c m) -> p c m", c=4) for h in range(2)]

            # ---- filler work units (generators; yield ~est ns per PE slot) ----
            def qk_gemm_unit(tch, mt):
                """q or k m-tile GEMM (fp8 DoubleRow) + bias move to qkt."""
                acc = gem.tile([128, 512], F32, tag="gacc", name="gacc")
                for cp in range(NCT // 2):
                    h, lp = divmod(cp, 2)
                    nc.tensor.matmul(
                        acc[:],
                        w8v[h][:, 2 * lp:2 * lp + 2, mt * 128:(mt + 1) * 128],
                        x8v[tch][h][:, 2 * lp:2 * lp + 2, :],
                        start=(cp == 0), stop=(cp == NCT // 2 - 1),
                        perf_mode=DR,
                    )
                    yield MM_NS
                nc.vector.tensor_scalar_add(
                    qkt[mt][tch][:], acc[:], bqkv_sb[:, mt:mt + 1],
                )

            def v_gemm_unit(tch, i):
                """v for t-tile 4*tch+i, produced directly in [t, m] layout."""
                tt = 4 * tch + i
                acc = gem.tile([128, 512], F32, tag="gacc", name="vacc")
                for ct in range(NCT):
                    wv_src = wva if ct < 4 else wvb
                    nc.tensor.matmul(
                        acc[:, 0:256],
                        xv[tch][i // 2][:, i % 2, ct, :],
                        wv_src[:, (ct % 4) * 256:(ct % 4 + 1) * 256],
                        start=(ct == 0), stop=(ct == NCT - 1),
                    )
                    yield MM_NS / 2
                dst = v_sb[:, tt * HPC * VW:(tt + 1) * HPC * VW].rearrange(
                    "p (h e) -> p h e", h=HPC)[:, :, 0:D]
                src = acc[:, 0:256].rearrange("p (h d) -> p h d", h=HPC)
                nc.vector.tensor_copy(dst, src)   # gpsimd can't read PSUM

            def proj_unit(tch, i):
                """proj for t-tile 4*tch+i (all 4 heads), y write-out."""
                tt = 4 * tch + i
                yt = yst.tile([128, C], F16, tag="ytile", name="yt")
                for cc in range(2):
                    acc = gem.tile([128, 512], F32, tag="gacc", name="pacc")
                    for kt in range(2):
                        nc.tensor.matmul(
                            acc[:],
                            on_sb[:, kt * T + tt * 128: kt * T + tt * 128 + 128],
                            wp_sb[:, kt * C + cc * 512: kt * C + cc * 512 + 512],
                            start=(kt == 0), stop=(kt == 1),
                        )
                        yield MM_NS
                    nc.vector.tensor_copy(yt[:, cc * 512:(cc + 1) * 512], acc[:])
                nc.gpsimd.dma_start(y.ap()[tt * 128:(tt + 1) * 128, :], yt[:])

            # epilogue chunk (tch=3): kt-outer, split into a kt0 half (needs
            # only the second-to-last phase's heads, so it runs while the
            # last finish()'s reciprocal chain occupies the vector engine)
            # and a kt1 half. Unit 0 holds its accs in gem across the final
            # attention phase; units 1-2 take both halves of the (by then
            # idle) score-PSUM buffers at the tail; unit 3 reuses gem.
            c3accs = {}

            def proj3_kt0(i):
                tt = 12 + i
                if i == 0 or i == 3:
                    accs = [gem.tile([128, 512], F32, tag="gacc", name="pacc")[:]
                            for _ in range(2)]
                else:
                    big = psg.tile([128, 1024], F32, tag="sG", name="pacc")
                    accs = [big[:, 0:512], big[:, 512:1024]]
                for cc in range(2):
                    nc.tensor.matmul(
                        accs[cc],
                        on_sb[:, tt * 128: tt * 128 + 128],
                        wp_sb[:, cc * 512: cc * 512 + 512],
                        start=True, stop=False,
                    )
                    yield MM_NS
                c3accs[i] = accs

            def proj3_kt1(i):
                tt = 12 + i
                accs = c3accs[i]
                yt = yst.tile([128, C], F16, tag="ytile", name="yt")
                for cc in range(2):
                    nc.tensor.matmul(
                        accs[cc],
                        on_sb[:, T + tt * 128: T + tt * 128 + 128],
                        wp_sb[:, C + cc * 512: C + cc * 512 + 512],
                        start=False, stop=True,
                    )
                    yield MM_NS
                # scalar+vector both idle at the tail: one copy each
                nc.scalar.copy(yt[:, 0:512], accs[0])
                nc.vector.tensor_copy(yt[:, 512:1024], accs[1])
                eng = (nc.gpsimd, nc.sync, nc.scalar, nc.sync)[i]
                eng.dma_start(y.ap()[tt * 128:(tt + 1) * 128, :], yt[:])

            # Two filler queues: bfifo (qkv; hard deadline = its chunk's
            # attention) drains at exactly the deadline rate, cfifo (proj; no
            # deadline) backfills the rest of each slab's tensor capacity so
            # proj doesn't pile up after the last exp.
            bfifo, cfifo = [], []
            for tch in range(TCH):
                bfifo.append((('k', tch, 0), qk_gemm_unit(tch, 2)))
                bfifo.append((('q', tch, 0), qk_gemm_unit(tch, 0)))
                for i in range(4):
                    bfifo.append((('v', tch, i), v_gemm_unit(tch, i)))
                bfifo.append((('k', tch, 1), qk_gemm_unit(tch, 3)))
                bfifo.append((('q', tch, 1), qk_gemm_unit(tch, 1)))

            bpos, cpos = [0], [0]
            done = set()

            def step_front(fifo, pos):
                """Advance the head unit by one PE slot; returns est ns."""
                tag, gen = fifo[pos[0]]
                try:
                    return next(gen)
                except StopIteration:
                    done.add(tag)
                    pos[0] += 1
                    return 0.0

            def pull_b(ns):
                spent = 0.0
                while spent < ns and bpos[0] < len(bfifo):
                    spent += step_front(bfifo, bpos)

            def pull_c(ns):
                spent = 0.0
                while spent < ns and cpos[0] < len(cfifo):
                    spent += step_front(cfifo, cpos)

            def drain(tags):
                while not tags.issubset(done) and bpos[0] < len(bfifo):
                    step_front(bfifo, bpos)

            # ---- attention backbone: half-slabs of 1 s-tile x 512 t, 2 heads ----
            SCL = float(SCALE / (W8 * W8))   # q,k carry the fp8 32x pre-scale

            class AttnPhase:
                def __init__(self, tch, hp):
                    self.tch, self.hp = tch, hp
                    self.n_half = 4 * (tch + 1)
                    self.acc = None
                    self.pq = []          # FIFO of (p, c0) awaiting PV

                def req(self):
                    r = set()
                    for c in range(self.tch + 1):
                        r |= {('k', c, self.hp), ('q', c, self.hp)}
                    return r

                def qk_half(self, st):
                    """Scores + exp + causal handling for s-tile st; keeps p."""
                    tch, hp = self.tch, self.hp
                    r = st - 4 * tch          # >=0 means diagonal half-slab
                    diag = (r >= 0)
                    c0 = 128 * r if diag else 0
                    kt = qkt[2 + hp][st // 4]         # k m-tile, s-chunk
                    qt = qkt[hp][tch]                 # q m-tile, this chunk
                    ko = (st % 4) * 128
                    sg = psg.tile([128, 1024], F32, tag="sG", name="sg")
                    p = att.tile([128, 1024], F16, tag="p", name="p")
                    nc.tensor.matmul(
                        sg[:, c0:512],
                        kt[0:64, ko:ko + 128],
                        qt[0:64, c0:512],
                        start=True, stop=True, tile_position=(0, 0),
                    )
                    nc.tensor.matmul(
                        sg[:, 512 + c0:1024],
                        kt[64:128, ko:ko + 128],
                        qt[64:128, c0:512],
                        start=True, stop=True, tile_position=(64, 0),
                    )
                    if not diag or r == 0:
                        nc.scalar.activation(p[:], sg[:], EXP, scale=SCL)
                    else:
                        pview = p[:].rearrange("p (h u) -> p h u", h=2)
                        sgview = sg[:].rearrange("p (h u) -> p h u", h=2)
                        nc.scalar.activation(pview[:, :, c0:512],
                                             sgview[:, :, c0:512],
                                             EXP, scale=SCL)
                    if diag:
                        # triangular boundary block: 0/1 mask on vector (fast)
                        pview = p[:].rearrange("p (h u) -> p h u", h=2)
                        tri = mask_sb[:, 0:128]
                        for h in range(2):
                            blk = pview[:, h, c0:c0 + 128]
                            nc.vector.tensor_mul(blk, blk, tri)
                    self.pq.append((p, c0))

                def pv_half(self, st):
                    if self.acc is None:
                        self.acc = (
                            psacc.tile([128, 512], F32, tag="acc0", name="acc0"),
                            psacc.tile([128, 512], F32, tag="acc1", name="acc1"),
                        )
                    (p, c0), hp = self.pq.pop(0), self.hp
                    first, last = (st == 0), (st == self.n_half - 1)
                    # diagonal half-slabs stream only the valid col suffix;
                    # c0==0 on st==0 always, so `start` covers the full acc.
                    # stop lands on a partial range: fine on hw (stop is a
                    # sim-only bookkeeping flag) -> skip_group_check.
                    for h in range(2):
                        nc.tensor.matmul(
                            self.acc[h][:, c0:512],
                            v_sb[:, st * HPC * VW + (2 * hp + h) * VW:
                                 st * HPC * VW + (2 * hp + h) * VW + VW],
                            p[:, h * 512 + c0:(h + 1) * 512],
                            start=first, stop=last, skip_group_check=True,
                        )

                def finish(self):
                    # normalize: O_norm^T = O^T*(1/l), l on rows 64..127
                    for i in range(2):
                        a = 2 * self.hp + i   # head index in core
                        # full-tile recip: the custom-DVE op mishandles
                        # partition slices; rows 0..63 are garbage, unused
                        rl = rlp.tile([128, 512], F32, tag="rl", name="rl")
                        nc.vector.reciprocal_approx_fast(rl[:], self.acc[i][:])
                        po = (a % 2) * 64
                        dst = on_sb[po:po + 64, (a // 2) * T + self.tch * 512:
                                    (a // 2) * T + self.tch * 512 + 512]
                        nc.vector.tensor_mul(dst, self.acc[i][0:D, :],
                                             rl[64:128, :])

            # ---- driver: one half-slab stream with qkv/proj interleaved.
            # Program order per half-slab: QK(g+1) first (keeps the exp
            # stream fed; sg is double-buffered so it never waits on exp),
            # then filler to absorb the exp(g) latency, then PV(g).
            # per-half-slab budgets: B at its deadline rate (next chunk's
            # qkv spread over this chunk's half-slabs), C backfills.
            QUOTA_B = [300.0, 550.0, 450.0, 450.0, 200.0, 200.0, 0.0, 0.0]
            QUOTA_C = [0.0, 0.0, 0.0, 0.0, 250.0, 250.0, 250.0, 250.0]
            STAGE_AT = {(1, 0): 2, (2, 0): 3}
            phases = [AttnPhase(tch, hp) for tch in range(TCH) for hp in range(2)]
            drain(phases[0].req())
            phases[0].qk_half(0)
            for idx, ph in enumerate(phases):
                for g in range(ph.n_half):
                    if (idx, g) in STAGE_AT:
                        emit_stage(STAGE_AT[(idx, g)])
                    if g + 1 < ph.n_half:
                        ph.qk_half(g + 1)
                    elif idx + 1 < len(phases):
                        nxt = phases[idx + 1]
                        drain(nxt.req())
                        nxt.qk_half(0)
                    pull_b(QUOTA_B[idx])
                    pull_c(QUOTA_C[idx])
                    # v tile this PV contracts must already be emitted
                    drain({('v', g // 4, g % 4)})
                    ph.pv_half(g)
                if ph.tch == TCH - 1 and ph.hp == 1:
                    # emit all kt0 halves BEFORE finish(): the framework's
                    # on_sb hazard check is conservative, so any on_sb read
                    # emitted after finish() waits for its muls; these fill
                    # the PE while the reciprocal/mul chain runs on vector
                    cfifo.append((('c3k0', 0), proj3_kt0(0)))
                    cfifo.append((('c3k0', 1), proj3_kt0(1)))
                    cfifo.append((('c3k0', 2), proj3_kt0(2)))
                    while cpos[0] < len(cfifo):
                        step_front(cfifo, cpos)
                ph.finish()
                if ph.hp == 1 and ph.tch < TCH - 1:
                    for i in range(4):
                        cfifo.append((('c', ph.tch, i), proj_unit(ph.tch, i)))
                elif ph.tch == TCH - 1 and ph.hp == 1:
                    cfifo.append((('c3k1', 0), proj3_kt1(0)))
                    cfifo.append((('c3k1', 1), proj3_kt1(1)))
                    cfifo.append((('c3k1', 2), proj3_kt1(2)))
                    cfifo.append((('c3k0', 3), proj3_kt0(3)))
                    cfifo.append((('c3k1', 3), proj3_kt1(3)))
            while bpos[0] < len(bfifo):
                step_front(bfifo, bpos)
            while cpos[0] < len(cfifo):
                step_front(cfifo, cpos)

    nc.compile()
    return nc


def _causal_mask():
    """tri[p, j] = 1.0 if p <= j else 0.0 (diagonal boundary block)."""
    p = np.arange(128)[:, None]
    j = np.arange(128)[None, :]
    return (p <= j).astype(np.float32)


def _pack(M, n):
    """[n*128, S] -> [128, n*S] SBUF c-tile packing."""
    S = M.shape[1]
    return np.ascontiguousarray(
        M.reshape(n, 128, S).transpose(1, 0, 2).reshape(128, n * S))


def _pack_x8(Mt):
    """xT [C, T] -> [128, (tch, ct, 512)] chunk-major packing."""
    A = Mt.reshape(NCT, 128, TCH, 512)
    return np.ascontiguousarray(
        A.transpose(1, 2, 0, 3).reshape(128, TCH * NCT * 512))


def _pack_x16(Mt):
    """xT [C, T] -> [128, (tch, t-tile, ct, 128)] t-tile-major packing."""
    A = Mt.reshape(NCT, 128, TCH, 4, 128)
    return np.ascontiguousarray(
        A.transpose(1, 2, 3, 0, 4).reshape(128, TCH * NCT * 512))


def _in_maps(x, w_qkv, b_qkv, w_proj):
    F8NP = mybir.dt.np(F8)
    mask = _causal_mask()
    xp16 = [_pack_x16(x[b].T.astype(np.float16)) for b in range(B)]
    xp8 = [_pack_x8(x[b].T.astype(F8NP)) for b in range(B)]
    maps = []
    for core in range(N_CORES):
        b, hg = divmod(core, 4)
        h0 = hg * HPC                       # first global head of this core
        r0 = h0 * D                         # first q row
        q_w = w_qkv[r0:r0 + HPC * D]                    # [256, C]
        k_w = w_qkv[C + r0:C + r0 + HPC * D]
        v_w = w_qkv[2 * C + r0:2 * C + r0 + HPC * D]
        wqk8 = np.ascontiguousarray(
            (np.concatenate([q_w, k_w], axis=0) * W8).T).astype(F8NP)
        wvT = np.ascontiguousarray(v_w.T)                           # [C, 256]
        wpT = np.ascontiguousarray(w_proj[:, r0:r0 + HPC * D].T)    # [256, C]
        bqkv = np.ascontiguousarray(np.concatenate(
            [b_qkv[r0:r0 + HPC * D], b_qkv[C + r0:C + r0 + HPC * D]]
        ).reshape(4, 128).T) * W8                                    # [128,4]
        maps.append({
            "xp": xp16[b],
            "xp8": xp8[b],
            "wqk8": _pack(wqk8, NCT),
            "wvp": _pack(wvT.astype(np.float16), NCT),
            "wpp": _pack(wpT.astype(np.float16), 2),
            "bqkv": np.ascontiguousarray(bqkv),
            "mask": mask.astype(np.float16),
        })
    return maps


def kernel(x, w_qkv, b_qkv, w_proj, b_proj, _trace=False, _tmpdir=None):
    x = np.asarray(x, dtype=np.float32)
    w_qkv = np.asarray(w_qkv, dtype=np.float32)
    b_qkv = np.asarray(b_qkv, dtype=np.float32)
    w_proj = np.asarray(w_proj, dtype=np.float32)
    b_proj = np.asarray(b_proj, dtype=np.float32)

    if "nc" not in _CACHE:
        _CACHE["nc"] = _build()
    nc = _CACHE["nc"]

    maps = _in_maps(x, w_qkv, b_qkv, w_proj)
    kw = {}
    if _trace:
        kw = {"trace": True, "tmpdir": _tmpdir}
    res = run_bass_kernel_spmd(nc, maps, list(range(N_CORES)), **kw)

    # v-bias flows linearly through attention: fold w_proj @ b_v into the
    # output bias added on the host.
    b_eff = w_proj @ b_qkv[2 * C:3 * C] + b_proj
    out = np.empty((B, T, C), dtype=np.float32)
    for b in range(B):
        acc = res.results[4 * b]["y"].astype(np.float32)
        for hg in range(1, 4):
            acc = acc + res.results[4 * b + hg]["y"].astype(np.float32)
        out[b] = acc + b_eff[None, :]
    if _trace:
        return out, res
    return out

